# revision 7
# baseline (speedup 1.0000x reference)
"""Trainium2 Bass kernel for single-head full-dim attention (nn_CasualSelfAttention).

Reference math (B=4, S=4096, D=768, fp32):
    q = x @ Wq.T + bq ; k = x @ Wk.T + bk ; v = x @ Wv.T + bv
    att = softmax(q @ k.T * D**-0.5)        # no mask
    y = att @ v
    y = y.transpose(0,2,1).reshape(B,S,D)   # element permutation
    out = y @ Wc.T + bc

Sharding (8 cores): core c = 2*b + h handles batch b with ALL 4096 queries but
only its half of the keys/values (rows h*2048:(h+1)*2048). Each core produces a
partial unnormalized yT [768, 4096] (features x queries) plus partial softmax
sums, with the value bias folded in linearly (bv x partial_sums). A pairwise
ReduceScatter(add) chunked by feature rows hands core h the fully-reduced
feature slice [384*h : 384*h+384] for all queries — exactly the rows of y.T
that the permutation maps to output rows [2048*h : 2048*h+2048]. After
normalizing by the (also-reduced) sums, the flat buffer IS y_perm row-major,
and the final projection runs locally. The RS is split into 4 query-column
blocks so communication overlaps attention compute. Host pre-transposes/casts
weights and activations to bf16; matmuls are bf16 with fp32 accumulation,
softmax in fp32 (logits are bounded ~|1.8| so no max subtraction is needed).
"""

import numpy as np
import ml_dtypes

BF16 = ml_dtypes.bfloat16

B, S, D = 4, 4096, 768
SK = S // 2            # keys per core
P = 128
DT = D // P            # 6 feature tiles
KT = SK // P           # 16 key tiles
QC = 512               # query chunk width
NQC = S // QC          # 8 query chunks
NBLK = 4               # RS column blocks
BW = S // NBLK         # 1024 columns per block
CPB = BW // QC         # q-chunks per block (2)
FH = D // 2            # 384: feature rows per RS chunk
SCALE = float(D) ** -0.5
GROUPS = [[0, 1], [2, 3], [4, 5], [6, 7]]

_nc = None


def _build_program(phases="ABCDEF"):
    import concourse.bass as bass
    import concourse.mybir as mybir
    import concourse.tile as tile
    from concourse import bacc, bass_isa

    f32 = mybir.dt.float32
    bf16 = mybir.dt.bfloat16
    Ident = mybir.ActivationFunctionType.Identity
    Exp = mybir.ActivationFunctionType.Exp
    mult = mybir.AluOpType.mult
    add = mybir.AluOpType.add

    nc = bacc.Bacc(None, num_devices=8)

    xq = nc.declare_dram_parameter("xq", [S, D], bf16, isOutput=False)
    xkv = nc.declare_dram_parameter("xkv", [SK, D], bf16, isOutput=False)
    wqT = nc.declare_dram_parameter("wqT", [D, D], bf16, isOutput=False)
    wkT = nc.declare_dram_parameter("wkT", [D, D], bf16, isOutput=False)
    wvT = nc.declare_dram_parameter("wvT", [D, D], bf16, isOutput=False)
    wcT = nc.declare_dram_parameter("wcT", [D, D], bf16, isOutput=False)
    bq = nc.declare_dram_parameter("bq", [D, 1], f32, isOutput=False)
    bk = nc.declare_dram_parameter("bk", [D, 1], f32, isOutput=False)
    bv = nc.declare_dram_parameter("bv", [D, 1], f32, isOutput=False)
    bc = nc.declare_dram_parameter("bc", [1, D], f32, isOutput=False)
    out = nc.declare_dram_parameter("out", [SK, D], f32, isOutput=True)

    with tile.TileContext(nc) as tc:
        with tc.tile_pool(name="persist", bufs=1) as pp, \
             tc.tile_pool(name="dram", bufs=1, space="DRAM") as dram:
            # Per column block: rows 0:384 = feats 0:384, row 384 = partial
            # sums, rows 385:769 = feats 384:768, row 769 = partial sums.
            yTaug = [dram.tile([2 * (FH + 1), BW], f32, name=f"yTaug{b}", tag=f"yTaug{b}")
                     for b in range(NBLK)]
            rs_out = [dram.tile([FH + 1, BW], f32, name=f"rs_out{b}", tag=f"rs_out{b}")
                      for b in range(NBLK)]
            f_dram = dram.tile([SK, D], bf16)

            # persistent SBUF: kT/qT/v activations + biases + output weights
            kT_sb = [pp.tile([P, SK], bf16, name=f"kT{g}", tag=f"kT{g}") for g in range(DT)]
            qT_sb = [pp.tile([P, S], bf16, name=f"qT{g}", tag=f"qT{g}") for g in range(DT)]
            v_sb = [pp.tile([P, D], bf16, name=f"v{t}", tag=f"v{t}") for t in range(KT)]
            bq_sb = [pp.tile([P, 1], f32, name=f"bq{g}", tag=f"bq{g}") for g in range(DT)]
            bk_sb = [pp.tile([P, 1], f32, name=f"bk{g}", tag=f"bk{g}") for g in range(DT)]
            bv_sb = [pp.tile([P, 1], f32, name=f"bv{g}", tag=f"bv{g}") for g in range(DT)]
            for g in range(DT):
                nc.sync.dma_start(bq_sb[g][:], bq[g * P:(g + 1) * P, :])
                nc.sync.dma_start(bk_sb[g][:], bk[g * P:(g + 1) * P, :])
                nc.sync.dma_start(bv_sb[g][:], bv[g * P:(g + 1) * P, :])
            wc_sb = [pp.tile([P, D], bf16, name=f"wc{g}", tag=f"wc{g}") for g in range(DT)]
            for g in range(DT):
                nc.sync.dma_start(wc_sb[g][:], wcT[g * P:(g + 1) * P, :])
            bc_sb = pp.tile([1, D], f32, tag="bc_sb")
            nc.sync.dma_start(bc_sb[:], bc[:])
            bcb = pp.tile([P, D], f32, tag="bcb")
            nc.gpsimd.partition_broadcast(bcb[:], bc_sb[:])

            # ---- Phase A: kT [768, 2048] and v [2048, 768] from xkv ----
            if "A" in phases:
                with tc.tile_pool(name="pA", bufs=1) as pa, \
                     tc.tile_pool(name="psA", bufs=2, space="PSUM") as psa:
                    xkvT = pa.tile([P, DT, SK], bf16, tag="xkvT")
                    nc.scalar.dma_start_transpose(xkvT[:], xkv[:])
                    wk_sb = [pa.tile([P, D], bf16, name=f"wk{g}", tag=f"wk{g}") for g in range(DT)]
                    wv_sb = [pa.tile([P, D], bf16, name=f"wv{g}", tag=f"wv{g}") for g in range(DT)]
                    for g in range(DT):
                        nc.sync.dma_start(wk_sb[g][:], wkT[g * P:(g + 1) * P, :])
                        nc.sync.dma_start(wv_sb[g][:], wvT[g * P:(g + 1) * P, :])
                    for go in range(DT):
                        for c in range(SK // QC):
                            ps = psa.tile([P, QC], f32, tag="pk")
                            for gi in range(DT):
                                nc.tensor.matmul(
                                    ps[:], wk_sb[gi][:, go * P:(go + 1) * P],
                                    xkvT[:, gi, c * QC:(c + 1) * QC],
                                    start=(gi == 0), stop=(gi == DT - 1))
                            nc.scalar.activation(kT_sb[go][:, c * QC:(c + 1) * QC],
                                                 ps[:], Ident, bias=bk_sb[go][:], scale=1.0)
                    for t in range(KT):
                        for half in range(2):
                            ps = psa.tile([P, FH], f32, tag="pv")
                            for gi in range(DT):
                                nc.tensor.matmul(
                                    ps[:], xkvT[:, gi, t * P:(t + 1) * P],
                                    wv_sb[gi][:, half * FH:(half + 1) * FH],
                                    start=(gi == 0), stop=(gi == DT - 1))
                            nc.vector.tensor_copy(v_sb[t][:, half * FH:(half + 1) * FH], ps[:])

            # ---- Phase B: qT [768, 4096] from xq ----
            if "B" in phases:
                with tc.tile_pool(name="pB", bufs=1) as pb, \
                     tc.tile_pool(name="psB", bufs=2, space="PSUM") as psb:
                    xqT = pb.tile([P, DT, S], bf16, tag="xqT")
                    nc.scalar.dma_start_transpose(xqT[:], xq[:])
                    wq_sb = [pb.tile([P, D], bf16, name=f"wq{g}", tag=f"wq{g}") for g in range(DT)]
                    for g in range(DT):
                        nc.sync.dma_start(wq_sb[g][:], wqT[g * P:(g + 1) * P, :])
                    for go in range(DT):
                        for c in range(NQC):
                            ps = psb.tile([P, QC], f32, tag="pq")
                            for gi in range(DT):
                                nc.tensor.matmul(
                                    ps[:], wq_sb[gi][:, go * P:(go + 1) * P],
                                    xqT[:, gi, c * QC:(c + 1) * QC],
                                    start=(gi == 0), stop=(gi == DT - 1))
                            nc.scalar.activation(qT_sb[go][:, c * QC:(c + 1) * QC],
                                                 ps[:], Ident, bias=bq_sb[go][:], scale=1.0)

            # ---- Phase C: attention; write yTaug; chunked RS + normalize ----
            if "C" in phases:
                with tc.tile_pool(name="pC", bufs=2) as pc, \
                     tc.tile_pool(name="pE", bufs=2) as pe, \
                     tc.tile_pool(name="psC", bufs=1, space="PSUM") as psc:
                    for qc in range(NQC):
                        blk, col = divmod(qc, CPB)
                        sums_acc = pc.tile([P, QC], f32, tag="sums_acc")
                        nc.vector.memset(sums_acc[:], 0.0)
                        ypsum = [psc.tile([P, QC], f32, name=f"y{e}", tag=f"y{e}", bufs=1)
                                 for e in range(DT)]
                        a_tiles = {}
                        for kt in range(KT):
                            aps = psc.tile([P, QC], f32, tag="att", bufs=2)
                            for gi in range(DT):
                                nc.tensor.matmul(
                                    aps[:], kT_sb[gi][:, kt * P:(kt + 1) * P],
                                    qT_sb[gi][:, qc * QC:(qc + 1) * QC],
                                    start=(gi == 0), stop=(gi == DT - 1))
                            # software pipeline: y-matmuls for kt-1 issue while
                            # the exp for kt is still on the scalar engine
                            if kt > 0:
                                for e in range(DT):
                                    nc.tensor.matmul(
                                        ypsum[e][:], v_sb[kt - 1][:, e * P:(e + 1) * P],
                                        a_tiles[kt - 1][:],
                                        start=(kt - 1 == 0), stop=False)
                            a_sb = pc.tile([P, QC], bf16, tag="a_sb", bufs=4)
                            a_tiles[kt] = a_sb
                            nc.scalar.activation(a_sb[:], aps[:], Exp, scale=SCALE)
                            nc.vector.tensor_add(sums_acc[:], sums_acc[:], a_sb[:])
                        for e in range(DT):
                            nc.tensor.matmul(
                                ypsum[e][:], v_sb[KT - 1][:, e * P:(e + 1) * P],
                                a_tiles[KT - 1][:],
                                start=False, stop=True)
                        # sums over the key axis, result on all partitions
                        sbc = pc.tile([P, QC], f32, tag="sbc")
                        nc.gpsimd.partition_all_reduce(
                            sbc[:], sums_acc[:], channels=P,
                            reduce_op=bass_isa.ReduceOp.add)
                        yb = yTaug[blk]
                        nc.sync.dma_start(yb[FH:FH + 1, col * QC:(col + 1) * QC], sbc[0:1, :])
                        nc.sync.dma_start(yb[2 * FH + 1:2 * FH + 2, col * QC:(col + 1) * QC], sbc[0:1, :])
                        for e in range(DT):
                            yt_sb = pc.tile([P, QC], f32, tag="yt_sb", bufs=3)
                            # (sums_bcast * bv[e]) + ypsum  — folds the value bias
                            nc.vector.scalar_tensor_tensor(
                                yt_sb[:], sbc[:], bv_sb[e][:], ypsum[e][:], mult, add)
                            row = e * P if e < 3 else (FH + 1) + (e - 3) * P
                            nc.sync.dma_start(
                                yb[row:row + P, col * QC:(col + 1) * QC], yt_sb[:])

                        if col == CPB - 1 and "D" in phases:
                            # block complete: reduce-scatter it, then normalize
                            nc.gpsimd.collective_compute(
                                "ReduceScatter", mybir.AluOpType.add,
                                replica_groups=GROUPS,
                                ins=[yTaug[blk].opt()], outs=[rs_out[blk].opt()])
                            if "E" in phases:
                                s_row = pe.tile([1, BW], f32, tag="s_row")
                                nc.sync.dma_start(s_row[:], rs_out[blk][FH:FH + 1, :])
                                rec = pe.tile([1, BW], f32, tag="rec")
                                nc.vector.reciprocal(rec[:], s_row[:])
                                rbc = pe.tile([P, BW], f32, tag="rbc")
                                nc.gpsimd.partition_broadcast(rbc[:], rec[:])
                                f_view = f_dram[:].rearrange("a b -> (a b)").rearrange(
                                    "(x c) -> x c", c=S)
                                for r in range(FH // P):
                                    fr = pe.tile([P, BW], f32, tag="fr", bufs=2)
                                    nc.sync.dma_start(fr[:], rs_out[blk][r * P:(r + 1) * P, :])
                                    fn = pe.tile([P, BW], bf16, tag="fn", bufs=2)
                                    nc.vector.tensor_mul(fn[:], fr[:], rbc[:])
                                    nc.sync.dma_start(
                                        f_view[r * P:(r + 1) * P, blk * BW:(blk + 1) * BW], fn[:])

            # ---- Phase F: out = y_perm @ Wc.T + bc ----
            if "F" in phases:
                with tc.tile_pool(name="pF", bufs=1) as pf, \
                     tc.tile_pool(name="psF", bufs=2, space="PSUM") as psf:
                    for t in range(SK // P):
                        fT = pf.tile([P, DT, P], bf16, tag="fT", bufs=3)
                        nc.scalar.dma_start_transpose(fT[:], f_dram[t * P:(t + 1) * P, :])
                        po = psf.tile([P, QC], f32, tag="po")
                        po2 = psf.tile([P, D - QC], f32, tag="po2")
                        for gi in range(DT):
                            nc.tensor.matmul(po[:], fT[:, gi, :], wc_sb[gi][:, 0:QC],
                                             start=(gi == 0), stop=(gi == DT - 1))
                            nc.tensor.matmul(po2[:], fT[:, gi, :], wc_sb[gi][:, QC:D],
                                             start=(gi == 0), stop=(gi == DT - 1))
                        o_sb = pf.tile([P, D], f32, tag="o_sb", bufs=3)
                        nc.vector.tensor_add(o_sb[:, 0:QC], po[:], bcb[:, 0:QC])
                        nc.vector.tensor_add(o_sb[:, QC:D], po2[:], bcb[:, QC:D])
                        nc.sync.dma_start(out[t * P:(t + 1) * P, :], o_sb[:])

    return nc


def _get_nc():
    global _nc
    if _nc is None:
        _nc = _build_program()
        _nc.finalize()
    return _nc


def _prep_in_maps(x, Wq, bq, Wk, bk, Wv, bv, Wc, bc):
    x = np.asarray(x, dtype=np.float32)
    wqT = np.ascontiguousarray(np.asarray(Wq, np.float32).T).astype(BF16)
    wkT = np.ascontiguousarray(np.asarray(Wk, np.float32).T).astype(BF16)
    wvT = np.ascontiguousarray(np.asarray(Wv, np.float32).T).astype(BF16)
    wcT = np.ascontiguousarray(np.asarray(Wc, np.float32).T).astype(BF16)
    bqc = np.asarray(bq, np.float32).reshape(D, 1).copy()
    bkc = np.asarray(bk, np.float32).reshape(D, 1).copy()
    bvc = np.asarray(bv, np.float32).reshape(D, 1).copy()
    bcc = np.asarray(bc, np.float32).reshape(1, D).copy()
    in_maps = []
    for c in range(8):
        b, h = divmod(c, 2)
        xb = x[b].astype(BF16)
        in_maps.append({
            "xq": xb,
            "xkv": np.ascontiguousarray(xb[h * SK:(h + 1) * SK]),
            "wqT": wqT, "wkT": wkT, "wvT": wvT, "wcT": wcT,
            "bq": bqc, "bk": bkc, "bv": bvc, "bc": bcc,
        })
    return in_maps


def _assemble(results):
    out = np.empty((B, S, D), dtype=np.float32)
    for c in range(8):
        b, h = divmod(c, 2)
        out[b, h * SK:(h + 1) * SK, :] = results[c]["out"]
    return out


def run_on_hw(trace=False, **inputs):
    from concourse.bass_utils import run_bass_kernel_spmd
    nc = _get_nc()
    in_maps = _prep_in_maps(**inputs)
    res = run_bass_kernel_spmd(nc, in_maps, list(range(8)), trace=trace)
    return _assemble(res.results), res


def kernel(**inputs):
    out, _ = run_on_hw(trace=False, **inputs)
    return out


# revision 8
# speedup vs baseline: 1.0278x; 1.0278x over previous
"""Trainium2 Bass kernel for single-head full-dim attention (nn_CasualSelfAttention).

Reference math (B=4, S=4096, D=768, fp32):
    q = x @ Wq.T + bq ; k = x @ Wk.T + bk ; v = x @ Wv.T + bv
    att = softmax(q @ k.T * D**-0.5)        # no mask
    y = att @ v
    y = y.transpose(0,2,1).reshape(B,S,D)   # element permutation
    out = y @ Wc.T + bc

Sharding (8 cores): core c = 2*b + h handles batch b with ALL 4096 queries but
only its half of the keys/values (rows h*2048:(h+1)*2048). Each core produces a
partial unnormalized yT [768, 4096] (features x queries) plus partial softmax
sums, with the value bias folded in linearly (bv x partial_sums). A pairwise
ReduceScatter(add) chunked by feature rows hands core h the fully-reduced
feature slice [384*h : 384*h+384] for all queries — exactly the rows of y.T
that the permutation maps to output rows [2048*h : 2048*h+2048]. After
normalizing by the (also-reduced) sums, the flat buffer IS y_perm row-major,
and the final projection runs locally. The RS is split into 4 query-column
blocks so communication overlaps attention compute. Host pre-transposes/casts
weights and activations to bf16; matmuls are bf16 with fp32 accumulation,
softmax in fp32 (logits are bounded ~|1.8| so no max subtraction is needed).
"""

import numpy as np
import ml_dtypes

BF16 = ml_dtypes.bfloat16

B, S, D = 4, 4096, 768
SK = S // 2            # keys per core
P = 128
DT = D // P            # 6 feature tiles
KT = SK // P           # 16 key tiles
QC = 512               # query chunk width
NQC = S // QC          # 8 query chunks
NBLK = 4               # RS column blocks
BW = S // NBLK         # 1024 columns per block
CPB = BW // QC         # q-chunks per block (2)
FH = D // 2            # 384: feature rows per RS chunk
SCALE = float(D) ** -0.5
GROUPS = [[0, 1], [2, 3], [4, 5], [6, 7]]

_nc = None


def _build_program(phases="ABCDEF"):
    import concourse.bass as bass
    import concourse.mybir as mybir
    import concourse.tile as tile
    from concourse import bacc, bass_isa

    f32 = mybir.dt.float32
    bf16 = mybir.dt.bfloat16
    Ident = mybir.ActivationFunctionType.Identity
    Exp = mybir.ActivationFunctionType.Exp
    mult = mybir.AluOpType.mult
    add = mybir.AluOpType.add

    nc = bacc.Bacc(None, num_devices=8)

    xq = nc.declare_dram_parameter("xq", [S, D], bf16, isOutput=False)
    xkv = nc.declare_dram_parameter("xkv", [SK, D], bf16, isOutput=False)
    wqT = nc.declare_dram_parameter("wqT", [D, D], bf16, isOutput=False)
    wkT = nc.declare_dram_parameter("wkT", [D, D], bf16, isOutput=False)
    wvT = nc.declare_dram_parameter("wvT", [D, D], bf16, isOutput=False)
    wcT = nc.declare_dram_parameter("wcT", [D, D], bf16, isOutput=False)
    bq = nc.declare_dram_parameter("bq", [D, 1], f32, isOutput=False)
    bk = nc.declare_dram_parameter("bk", [D, 1], f32, isOutput=False)
    bv = nc.declare_dram_parameter("bv", [D, 1], f32, isOutput=False)
    bc = nc.declare_dram_parameter("bc", [1, D], f32, isOutput=False)
    out = nc.declare_dram_parameter("out", [SK, D], f32, isOutput=True)

    with tile.TileContext(nc) as tc:
        with tc.tile_pool(name="persist", bufs=1) as pp, \
             tc.tile_pool(name="dram", bufs=1, space="DRAM") as dram:
            # Per column block: rows 0:384 = feats 0:384, row 384 = partial
            # sums, rows 385:769 = feats 384:768, row 769 = partial sums.
            yTaug = [dram.tile([2 * (FH + 1), BW], f32, name=f"yTaug{b}", tag=f"yTaug{b}")
                     for b in range(NBLK)]
            rs_out = [dram.tile([FH + 1, BW], f32, name=f"rs_out{b}", tag=f"rs_out{b}")
                      for b in range(NBLK)]
            f_dram = dram.tile([SK, D], bf16)

            # persistent SBUF: kT/qT/v activations + biases + output weights
            kT_sb = [pp.tile([P, SK], bf16, name=f"kT{g}", tag=f"kT{g}") for g in range(DT)]
            qT_sb = [pp.tile([P, S], bf16, name=f"qT{g}", tag=f"qT{g}") for g in range(DT)]
            v_sb = [pp.tile([P, D], bf16, name=f"v{t}", tag=f"v{t}") for t in range(KT)]
            bq_sb = [pp.tile([P, 1], f32, name=f"bq{g}", tag=f"bq{g}") for g in range(DT)]
            bk_sb = [pp.tile([P, 1], f32, name=f"bk{g}", tag=f"bk{g}") for g in range(DT)]
            bv_sb = [pp.tile([P, 1], f32, name=f"bv{g}", tag=f"bv{g}") for g in range(DT)]
            for g in range(DT):
                nc.sync.dma_start(bq_sb[g][:], bq[g * P:(g + 1) * P, :])
                nc.sync.dma_start(bk_sb[g][:], bk[g * P:(g + 1) * P, :])
                nc.sync.dma_start(bv_sb[g][:], bv[g * P:(g + 1) * P, :])
            wc_sb = [pp.tile([P, D], bf16, name=f"wc{g}", tag=f"wc{g}") for g in range(DT)]
            for g in range(DT):
                nc.sync.dma_start(wc_sb[g][:], wcT[g * P:(g + 1) * P, :])
            bc_sb = pp.tile([1, D], f32, tag="bc_sb")
            nc.sync.dma_start(bc_sb[:], bc[:])
            bcb = pp.tile([P, D], f32, tag="bcb")
            nc.gpsimd.partition_broadcast(bcb[:], bc_sb[:])

            # ---- Phase A: kT [768, 2048] and v [2048, 768] from xkv ----
            if "A" in phases:
                with tc.tile_pool(name="pA", bufs=1) as pa, \
                     tc.tile_pool(name="psA", bufs=2, space="PSUM") as psa:
                    xkvT_c = [pa.tile([P, DT, QC], bf16, name=f"xkvT{c}", tag=f"xkvT{c}")
                              for c in range(SK // QC)]
                    for c in range(SK // QC):
                        nc.scalar.dma_start_transpose(xkvT_c[c][:], xkv[c * QC:(c + 1) * QC, :])
                    wk_sb = [pa.tile([P, D], bf16, name=f"wk{g}", tag=f"wk{g}") for g in range(DT)]
                    wv_sb = [pa.tile([P, D], bf16, name=f"wv{g}", tag=f"wv{g}") for g in range(DT)]
                    for g in range(DT):
                        nc.sync.dma_start(wk_sb[g][:], wkT[g * P:(g + 1) * P, :])
                        nc.sync.dma_start(wv_sb[g][:], wvT[g * P:(g + 1) * P, :])
                    for go in range(DT):
                        for c in range(SK // QC):
                            ps = psa.tile([P, QC], f32, tag="pk")
                            for gi in range(DT):
                                nc.tensor.matmul(
                                    ps[:], wk_sb[gi][:, go * P:(go + 1) * P],
                                    xkvT_c[c][:, gi, :],
                                    start=(gi == 0), stop=(gi == DT - 1))
                            nc.vector.tensor_scalar_add(
                                kT_sb[go][:, c * QC:(c + 1) * QC], ps[:], bk_sb[go][:])
                    for t in range(KT):
                        for half in range(2):
                            ps = psa.tile([P, FH], f32, tag="pv")
                            for gi in range(DT):
                                nc.tensor.matmul(
                                    ps[:], xkvT_c[t // 4][:, gi, (t % 4) * P:(t % 4 + 1) * P],
                                    wv_sb[gi][:, half * FH:(half + 1) * FH],
                                    start=(gi == 0), stop=(gi == DT - 1))
                            nc.vector.tensor_copy(v_sb[t][:, half * FH:(half + 1) * FH], ps[:])

            # ---- Phase B: qT [768, 4096] from xq ----
            if "B" in phases:
                with tc.tile_pool(name="pB", bufs=1) as pb, \
                     tc.tile_pool(name="psB", bufs=2, space="PSUM") as psb:
                    xqT_c = [pb.tile([P, DT, QC], bf16, name=f"xqT{c}", tag=f"xqT{c}")
                             for c in range(NQC)]
                    for c in range(NQC):
                        nc.scalar.dma_start_transpose(xqT_c[c][:], xq[c * QC:(c + 1) * QC, :])
                    wq_sb = [pb.tile([P, D], bf16, name=f"wq{g}", tag=f"wq{g}") for g in range(DT)]
                    for g in range(DT):
                        nc.sync.dma_start(wq_sb[g][:], wqT[g * P:(g + 1) * P, :])
                    for go in range(DT):
                        for c in range(NQC):
                            ps = psb.tile([P, QC], f32, tag="pq")
                            for gi in range(DT):
                                nc.tensor.matmul(
                                    ps[:], wq_sb[gi][:, go * P:(go + 1) * P],
                                    xqT_c[c][:, gi, :],
                                    start=(gi == 0), stop=(gi == DT - 1))
                            nc.vector.tensor_scalar_add(
                                qT_sb[go][:, c * QC:(c + 1) * QC], ps[:], bq_sb[go][:])

            # ---- Phase C: attention; write yTaug; chunked RS + normalize ----
            if "C" in phases:
                with tc.tile_pool(name="pC", bufs=2) as pc, \
                     tc.tile_pool(name="pE", bufs=2) as pe, \
                     tc.tile_pool(name="psC", bufs=1, space="PSUM") as psc:
                    f_view = f_dram[:].rearrange("a b -> (a b)").rearrange(
                        "(x c) -> x c", c=S)

                    def emit_norm(b):
                        s_row = pe.tile([1, BW], f32, tag="s_row", name="s_row")
                        nc.sync.dma_start(s_row[:], rs_out[b][FH:FH + 1, :])
                        rec = pe.tile([1, BW], f32, tag="rec", name="rec")
                        nc.vector.reciprocal(rec[:], s_row[:])
                        rbc = pe.tile([P, BW], f32, tag="rbc", name="rbc")
                        nc.gpsimd.partition_broadcast(rbc[:], rec[:])
                        for r in range(FH // P):
                            fr = pe.tile([P, BW], f32, tag="fr", bufs=2, name="fr")
                            nc.sync.dma_start(fr[:], rs_out[b][r * P:(r + 1) * P, :])
                            fn = pe.tile([P, BW], bf16, tag="fn", bufs=2, name="fn")
                            nc.vector.tensor_mul(fn[:], fr[:], rbc[:])
                            nc.sync.dma_start(
                                f_view[r * P:(r + 1) * P, b * BW:(b + 1) * BW], fn[:])

                    for qc in range(NQC):
                        blk, col = divmod(qc, CPB)
                        sums_acc = pc.tile([P, QC], f32, tag="sums_acc")
                        nc.vector.memset(sums_acc[:], 0.0)
                        ypsum = [psc.tile([P, QC], f32, name=f"y{e}", tag=f"y{e}", bufs=1)
                                 for e in range(DT)]
                        a_tiles = {}
                        for kt in range(KT):
                            aps = psc.tile([P, QC], f32, tag="att", bufs=2)
                            for gi in range(DT):
                                nc.tensor.matmul(
                                    aps[:], kT_sb[gi][:, kt * P:(kt + 1) * P],
                                    qT_sb[gi][:, qc * QC:(qc + 1) * QC],
                                    start=(gi == 0), stop=(gi == DT - 1))
                            # software pipeline: y-matmuls for kt-1 issue while
                            # the exp for kt is still on the scalar engine
                            if kt > 0:
                                for e in range(DT):
                                    nc.tensor.matmul(
                                        ypsum[e][:], v_sb[kt - 1][:, e * P:(e + 1) * P],
                                        a_tiles[kt - 1][:],
                                        start=(kt - 1 == 0), stop=False)
                            a_sb = pc.tile([P, QC], bf16, tag="a_sb", bufs=4)
                            a_tiles[kt] = a_sb
                            nc.scalar.activation(a_sb[:], aps[:], Exp, scale=SCALE)
                            nc.vector.tensor_add(sums_acc[:], sums_acc[:], a_sb[:])
                        for e in range(DT):
                            nc.tensor.matmul(
                                ypsum[e][:], v_sb[KT - 1][:, e * P:(e + 1) * P],
                                a_tiles[KT - 1][:],
                                start=False, stop=True)
                        # sums over the key axis, result on all partitions
                        sbc = pc.tile([P, QC], f32, tag="sbc")
                        nc.gpsimd.partition_all_reduce(
                            sbc[:], sums_acc[:], channels=P,
                            reduce_op=bass_isa.ReduceOp.add)
                        yb = yTaug[blk]
                        nc.sync.dma_start(yb[FH:FH + 1, col * QC:(col + 1) * QC], sbc[0:1, :])
                        nc.sync.dma_start(yb[2 * FH + 1:2 * FH + 2, col * QC:(col + 1) * QC], sbc[0:1, :])
                        for e in range(DT):
                            yt_sb = pc.tile([P, QC], f32, tag="yt_sb", bufs=3)
                            # (sums_bcast * bv[e]) + ypsum  — folds the value bias
                            nc.vector.scalar_tensor_tensor(
                                yt_sb[:], sbc[:], bv_sb[e][:], ypsum[e][:], mult, add)
                            row = e * P if e < 3 else (FH + 1) + (e - 3) * P
                            nc.sync.dma_start(
                                yb[row:row + P, col * QC:(col + 1) * QC], yt_sb[:])

                        if col == CPB - 1 and "D" in phases:
                            # block complete: reduce-scatter it. Normalization
                            # of block b-1 is emitted here (one block late) so
                            # the gpsimd partition_broadcast never sits in the
                            # queue blocking on an in-flight collective.
                            nc.gpsimd.collective_compute(
                                "ReduceScatter", mybir.AluOpType.add,
                                replica_groups=GROUPS,
                                ins=[yTaug[blk].opt()], outs=[rs_out[blk].opt()])
                            if "E" in phases and blk > 0:
                                emit_norm(blk - 1)

                    if "D" in phases and "E" in phases:
                        emit_norm(NBLK - 1)

            # ---- Phase F: out = y_perm @ Wc.T + bc ----
            if "F" in phases:
                with tc.tile_pool(name="pF", bufs=1) as pf, \
                     tc.tile_pool(name="psF", bufs=2, space="PSUM") as psf:
                    fTs = []
                    for t in range(SK // P):
                        fT = pf.tile([P, DT, P], bf16, name=f"fT{t}", tag=f"fT{t}")
                        nc.scalar.dma_start_transpose(fT[:], f_dram[t * P:(t + 1) * P, :])
                        fTs.append(fT)
                    for t in range(SK // P):
                        fT = fTs[t]
                        po = psf.tile([P, QC], f32, tag="po")
                        po2 = psf.tile([P, D - QC], f32, tag="po2")
                        for gi in range(DT):
                            nc.tensor.matmul(po[:], fT[:, gi, :], wc_sb[gi][:, 0:QC],
                                             start=(gi == 0), stop=(gi == DT - 1))
                            nc.tensor.matmul(po2[:], fT[:, gi, :], wc_sb[gi][:, QC:D],
                                             start=(gi == 0), stop=(gi == DT - 1))
                        o_sb = pf.tile([P, D], f32, tag="o_sb", bufs=3)
                        nc.vector.tensor_add(o_sb[:, 0:QC], po[:], bcb[:, 0:QC])
                        nc.vector.tensor_add(o_sb[:, QC:D], po2[:], bcb[:, QC:D])
                        nc.sync.dma_start(out[t * P:(t + 1) * P, :], o_sb[:])

    return nc


def _get_nc():
    global _nc
    if _nc is None:
        _nc = _build_program()
        _nc.finalize()
    return _nc


def _prep_in_maps(x, Wq, bq, Wk, bk, Wv, bv, Wc, bc):
    x = np.asarray(x, dtype=np.float32)
    wqT = np.ascontiguousarray(np.asarray(Wq, np.float32).T).astype(BF16)
    wkT = np.ascontiguousarray(np.asarray(Wk, np.float32).T).astype(BF16)
    wvT = np.ascontiguousarray(np.asarray(Wv, np.float32).T).astype(BF16)
    wcT = np.ascontiguousarray(np.asarray(Wc, np.float32).T).astype(BF16)
    bqc = np.asarray(bq, np.float32).reshape(D, 1).copy()
    bkc = np.asarray(bk, np.float32).reshape(D, 1).copy()
    bvc = np.asarray(bv, np.float32).reshape(D, 1).copy()
    bcc = np.asarray(bc, np.float32).reshape(1, D).copy()
    in_maps = []
    for c in range(8):
        b, h = divmod(c, 2)
        xb = x[b].astype(BF16)
        in_maps.append({
            "xq": xb,
            "xkv": np.ascontiguousarray(xb[h * SK:(h + 1) * SK]),
            "wqT": wqT, "wkT": wkT, "wvT": wvT, "wcT": wcT,
            "bq": bqc, "bk": bkc, "bv": bvc, "bc": bcc,
        })
    return in_maps


def _assemble(results):
    out = np.empty((B, S, D), dtype=np.float32)
    for c in range(8):
        b, h = divmod(c, 2)
        out[b, h * SK:(h + 1) * SK, :] = results[c]["out"]
    return out


def run_on_hw(trace=False, **inputs):
    from concourse.bass_utils import run_bass_kernel_spmd
    nc = _get_nc()
    in_maps = _prep_in_maps(**inputs)
    res = run_bass_kernel_spmd(nc, in_maps, list(range(8)), trace=trace)
    return _assemble(res.results), res


def kernel(**inputs):
    out, _ = run_on_hw(trace=False, **inputs)
    return out


# revision 11
# speedup vs baseline: 1.1000x; 1.0703x over previous
"""Trainium2 Bass kernel for single-head full-dim attention (nn_CasualSelfAttention).

Reference math (B=4, S=4096, D=768, fp32):
    q = x @ Wq.T + bq ; k = x @ Wk.T + bk ; v = x @ Wv.T + bv
    att = softmax(q @ k.T * D**-0.5)        # no mask
    y = att @ v
    y = y.transpose(0,2,1).reshape(B,S,D)   # element permutation
    out = y @ Wc.T + bc

Sharding (8 cores): core c = 2*b + h handles batch b with ALL 4096 queries but
only its half of the keys/values (rows h*2048:(h+1)*2048). Each core produces a
partial unnormalized yT [768, 4096] (features x queries) plus partial softmax
sums, with the value bias folded in linearly (bv x partial_sums). A pairwise
ReduceScatter(add) chunked by feature rows hands core h the fully-reduced
feature slice [384*h : 384*h+384] for all queries — exactly the rows of y.T
that the permutation maps to output rows [2048*h : 2048*h+2048]. After
normalizing by the (also-reduced) sums, the flat buffer IS y_perm row-major,
and the final projection runs locally. The RS is split into query-column
blocks (the last ones small) so communication overlaps attention compute and
the serial tail is short. Host pre-transposes/casts weights and activations to
bf16; matmuls are bf16 with fp32 accumulation, softmax in fp32 (logits are
bounded ~|1.8| so no max subtraction is needed).
"""

import numpy as np
import ml_dtypes

BF16 = ml_dtypes.bfloat16

B, S, D = 4, 4096, 768
SK = S // 2            # keys per core
P = 128
DT = D // P            # 6 feature tiles
KT = SK // P           # 16 key tiles
QC = 512               # query chunk width
NQC = S // QC          # 8 query chunks
BLOCKS = [(0, 3), (3, 3), (6, 1), (7, 1)]   # RS blocks as (start_qc, n_qc)
FH = D // 2            # 384: feature rows per RS chunk
SCALE = float(D) ** -0.5
GROUPS = [[0, 1], [2, 3], [4, 5], [6, 7]]

_nc = None


def _build_program(phases="ABCDEF"):
    import concourse.bass as bass
    import concourse.mybir as mybir
    import concourse.tile as tile
    from concourse import bacc

    f32 = mybir.dt.float32
    bf16 = mybir.dt.bfloat16
    Exp = mybir.ActivationFunctionType.Exp
    mult = mybir.AluOpType.mult
    add = mybir.AluOpType.add

    qc2blk = {}
    for bi, (s0, n) in enumerate(BLOCKS):
        for j in range(n):
            qc2blk[s0 + j] = (bi, j)

    nc = bacc.Bacc(None, num_devices=8)

    xq = nc.declare_dram_parameter("xq", [S, D], bf16, isOutput=False)
    xkv = nc.declare_dram_parameter("xkv", [SK, D], bf16, isOutput=False)
    wqT = nc.declare_dram_parameter("wqT", [D, D], bf16, isOutput=False)
    wkT = nc.declare_dram_parameter("wkT", [D, D], bf16, isOutput=False)
    wvT = nc.declare_dram_parameter("wvT", [D, D], bf16, isOutput=False)
    wcT = nc.declare_dram_parameter("wcT", [D, D], bf16, isOutput=False)
    bq = nc.declare_dram_parameter("bq", [D, 1], f32, isOutput=False)
    bk = nc.declare_dram_parameter("bk", [D, 1], f32, isOutput=False)
    bv = nc.declare_dram_parameter("bv", [D, 1], f32, isOutput=False)
    bc = nc.declare_dram_parameter("bc", [1, D], f32, isOutput=False)
    out = nc.declare_dram_parameter("out", [SK, D], f32, isOutput=True)

    def wload(dst, src):
        # [768, 768] row-major -> [128, 6, 768] with logical row g*128+p
        nc.sync.dma_start(dst[:], src[:].rearrange("(g p) d -> p g d", p=P))

    with tile.TileContext(nc) as tc:
        with tc.tile_pool(name="persist", bufs=1) as pp, \
             tc.tile_pool(name="dram", bufs=1, space="DRAM") as dram:
            # Per column block: rows 0:384 = feats 0:384, row 384 = partial
            # sums, rows 385:769 = feats 384:768, row 769 = partial sums.
            yTaug = [dram.tile([2 * (FH + 1), n * QC], f32, name=f"yTaug{b}", tag=f"yTaug{b}")
                     for b, (_, n) in enumerate(BLOCKS)]
            rs_out = [dram.tile([FH + 1, n * QC], f32, name=f"rs_out{b}", tag=f"rs_out{b}")
                      for b, (_, n) in enumerate(BLOCKS)]
            f_dram = dram.tile([SK, D], bf16)

            # persistent SBUF: kT/qT/v activations + biases + output weights
            kT_sb = [pp.tile([P, SK], bf16, name=f"kT{g}", tag=f"kT{g}") for g in range(DT)]
            qT_sb = [pp.tile([P, S], bf16, name=f"qT{g}", tag=f"qT{g}") for g in range(DT)]
            v_sb = [pp.tile([P, D], bf16, name=f"v{t}", tag=f"v{t}") for t in range(KT)]
            bq_sb = [pp.tile([P, 1], f32, name=f"bq{g}", tag=f"bq{g}") for g in range(DT)]
            bk_sb = [pp.tile([P, 1], f32, name=f"bk{g}", tag=f"bk{g}") for g in range(DT)]
            bv_sb = [pp.tile([P, 1], f32, name=f"bv{g}", tag=f"bv{g}") for g in range(DT)]
            ones_sb = pp.tile([P, P], f32, name="ones", tag="ones")
            nc.vector.memset(ones_sb[:], 1.0)
            for g in range(DT):
                nc.sync.dma_start(bq_sb[g][:], bq[g * P:(g + 1) * P, :])
                nc.sync.dma_start(bk_sb[g][:], bk[g * P:(g + 1) * P, :])
                nc.sync.dma_start(bv_sb[g][:], bv[g * P:(g + 1) * P, :])
            wc_sb = pp.tile([P, DT, D], bf16, tag="wc_sb")
            wload(wc_sb, wcT)
            bc_sb = pp.tile([1, D], f32, tag="bc_sb")
            nc.sync.dma_start(bc_sb[:], bc[:])
            bcb = pp.tile([P, D], f32, tag="bcb")
            nc.gpsimd.partition_broadcast(bcb[:], bc_sb[:])

            # ---- Phase A: kT [768, 2048] and v [2048, 768] from xkv ----
            if "A" in phases:
                with tc.tile_pool(name="pA", bufs=1) as pa, \
                     tc.tile_pool(name="psA", bufs=2, space="PSUM") as psa:
                    wk_sb = pa.tile([P, DT, D], bf16, tag="wk_sb")
                    wload(wk_sb, wkT)
                    wv_sb = pa.tile([P, DT, D], bf16, tag="wv_sb")
                    wload(wv_sb, wvT)
                    xkvT_c = [pa.tile([P, DT, QC], bf16, name=f"xkvT{c}", tag=f"xkvT{c}")
                              for c in range(SK // QC)]
                    for c in range(SK // QC):
                        nc.sync.dma_start_transpose(xkvT_c[c][:], xkv[c * QC:(c + 1) * QC, :])
                    for c in range(SK // QC):
                        for go in range(DT):
                            ps = psa.tile([P, QC], f32, tag="pk")
                            for gi in range(DT):
                                nc.tensor.matmul(
                                    ps[:], wk_sb[:, gi, go * P:(go + 1) * P],
                                    xkvT_c[c][:, gi, :],
                                    start=(gi == 0), stop=(gi == DT - 1))
                            nc.vector.tensor_scalar_add(
                                kT_sb[go][:, c * QC:(c + 1) * QC], ps[:], bk_sb[go][:])
                    for t in range(KT):
                        for half in range(2):
                            ps = psa.tile([P, FH], f32, tag="pv")
                            for gi in range(DT):
                                nc.tensor.matmul(
                                    ps[:], xkvT_c[t // 4][:, gi, (t % 4) * P:(t % 4 + 1) * P],
                                    wv_sb[:, gi, half * FH:(half + 1) * FH],
                                    start=(gi == 0), stop=(gi == DT - 1))
                            nc.vector.tensor_copy(v_sb[t][:, half * FH:(half + 1) * FH], ps[:])

            # ---- Phase B: qT [768, 4096] from xq ----
            if "B" in phases:
                with tc.tile_pool(name="pB", bufs=1) as pb, \
                     tc.tile_pool(name="psB", bufs=2, space="PSUM") as psb:
                    wq_sb = pb.tile([P, DT, D], bf16, tag="wq_sb")
                    wload(wq_sb, wqT)
                    xqT_c = [pb.tile([P, DT, QC], bf16, name=f"xqT{c}", tag=f"xqT{c}")
                             for c in range(NQC)]
                    for c in range(NQC):
                        nc.sync.dma_start_transpose(xqT_c[c][:], xq[c * QC:(c + 1) * QC, :])
                    for c in range(NQC):
                        for go in range(DT):
                            ps = psb.tile([P, QC], f32, tag="pq")
                            for gi in range(DT):
                                nc.tensor.matmul(
                                    ps[:], wq_sb[:, gi, go * P:(go + 1) * P],
                                    xqT_c[c][:, gi, :],
                                    start=(gi == 0), stop=(gi == DT - 1))
                            nc.vector.tensor_scalar_add(
                                qT_sb[go][:, c * QC:(c + 1) * QC], ps[:], bq_sb[go][:])

            # ---- Phase C: attention; write yTaug; chunked RS + normalize ----
            if "C" in phases:
                with tc.tile_pool(name="pC", bufs=2) as pc, \
                     tc.tile_pool(name="pE", bufs=2) as pe, \
                     tc.tile_pool(name="psC", bufs=1, space="PSUM") as psc:
                    f_view = f_dram[:].rearrange("a b -> (a b)").rearrange(
                        "(x c) -> x c", c=S)

                    MAXBW = max(n for _, n in BLOCKS) * QC

                    def emit_norm(b):
                        bw = BLOCKS[b][1] * QC
                        c0 = BLOCKS[b][0] * QC
                        s_row = pe.tile([1, MAXBW], f32, tag="s_row", name="s_row")
                        nc.sync.dma_start(s_row[:, :bw], rs_out[b][FH:FH + 1, :])
                        rec = pe.tile([1, MAXBW], f32, tag="rec", name="rec")
                        nc.vector.reciprocal(rec[:, :bw], s_row[:, :bw])
                        rbc = pe.tile([P, MAXBW], f32, tag="rbc", name="rbc")
                        nc.gpsimd.partition_broadcast(rbc[:, :bw], rec[:, :bw])
                        for r in range(FH // P):
                            fr = pe.tile([P, MAXBW], f32, tag="fr", bufs=2, name="fr")
                            nc.sync.dma_start(fr[:, :bw], rs_out[b][r * P:(r + 1) * P, :])
                            fn = pe.tile([P, MAXBW], bf16, tag="fn", bufs=2, name="fn")
                            nc.vector.tensor_mul(fn[:, :bw], fr[:, :bw], rbc[:, :bw])
                            nc.sync.dma_start(
                                f_view[r * P:(r + 1) * P, c0:c0 + bw], fn[:, :bw])

                    for qc in range(NQC):
                        blk, col = qc2blk[qc]
                        sums_acc = pc.tile([P, QC], f32, tag="sums_acc")
                        nc.vector.memset(sums_acc[:], 0.0)
                        ypsum = [psc.tile([P, QC], f32, name=f"y{e}", tag=f"y{e}", bufs=1)
                                 for e in range(DT)]
                        a_tiles = {}
                        for kt in range(KT):
                            aps = psc.tile([P, QC], f32, tag="att", bufs=2)
                            for gi in range(DT):
                                nc.tensor.matmul(
                                    aps[:], kT_sb[gi][:, kt * P:(kt + 1) * P],
                                    qT_sb[gi][:, qc * QC:(qc + 1) * QC],
                                    start=(gi == 0), stop=(gi == DT - 1))
                            # software pipeline: y-matmuls for kt-1 issue while
                            # the exp for kt is still on the scalar engine
                            if kt > 0:
                                for e in range(DT):
                                    nc.tensor.matmul(
                                        ypsum[e][:], v_sb[kt - 1][:, e * P:(e + 1) * P],
                                        a_tiles[kt - 1][:],
                                        start=(kt - 1 == 0), stop=False)
                            a_sb = pc.tile([P, QC], bf16, tag="a_sb", bufs=4)
                            a_tiles[kt] = a_sb
                            nc.scalar.activation(a_sb[:], aps[:], Exp, scale=SCALE)
                            nc.vector.tensor_add(sums_acc[:], sums_acc[:], a_sb[:])
                        for e in range(DT):
                            nc.tensor.matmul(
                                ypsum[e][:], v_sb[KT - 1][:, e * P:(e + 1) * P],
                                a_tiles[KT - 1][:],
                                start=False, stop=True)
                        # ones.T @ sums_acc both reduces across partitions and
                        # replicates the result onto all 128 partitions
                        sp = psc.tile([P, QC], f32, tag="att", bufs=2)
                        nc.tensor.matmul(sp[:], ones_sb[:], sums_acc[:], start=True, stop=True)
                        sbc = pc.tile([P, QC], f32, tag="sbc")
                        nc.vector.tensor_copy(sbc[:], sp[:])
                        yb = yTaug[blk]
                        nc.sync.dma_start(yb[FH:FH + 1, col * QC:(col + 1) * QC], sbc[0:1, :])
                        nc.sync.dma_start(yb[2 * FH + 1:2 * FH + 2, col * QC:(col + 1) * QC], sbc[0:1, :])
                        for e in range(DT):
                            yt_sb = pc.tile([P, QC], f32, tag="yt_sb", bufs=3)
                            # (sums_bcast * bv[e]) + ypsum  — folds the value bias
                            nc.vector.scalar_tensor_tensor(
                                yt_sb[:], sbc[:], bv_sb[e][:], ypsum[e][:], mult, add)
                            row = e * P if e < 3 else (FH + 1) + (e - 3) * P
                            nc.sync.dma_start(
                                yb[row:row + P, col * QC:(col + 1) * QC], yt_sb[:])

                        if col == BLOCKS[blk][1] - 1 and "D" in phases:
                            # block complete: reduce-scatter it. Normalization
                            # of block b-1 is emitted here (one block late) so
                            # nothing queues up behind an in-flight collective.
                            nc.gpsimd.collective_compute(
                                "ReduceScatter", mybir.AluOpType.add,
                                replica_groups=GROUPS,
                                ins=[yTaug[blk].opt()], outs=[rs_out[blk].opt()])
                            if "E" in phases and blk > 0:
                                emit_norm(blk - 1)

                    if "D" in phases and "E" in phases:
                        emit_norm(len(BLOCKS) - 1)

            # ---- Phase F: out = y_perm @ Wc.T + bc ----
            if "F" in phases:
                with tc.tile_pool(name="pF", bufs=1) as pf, \
                     tc.tile_pool(name="psF", bufs=2, space="PSUM") as psf:
                    fTs = []
                    for t in range(SK // P):
                        fT = pf.tile([P, DT, P], bf16, name=f"fT{t}", tag=f"fT{t}")
                        nc.sync.dma_start_transpose(fT[:], f_dram[t * P:(t + 1) * P, :])
                        fTs.append(fT)
                    for t in range(SK // P):
                        fT = fTs[t]
                        po = psf.tile([P, QC], f32, tag="po")
                        po2 = psf.tile([P, D - QC], f32, tag="po2")
                        for gi in range(DT):
                            nc.tensor.matmul(po[:], fT[:, gi, :], wc_sb[:, gi, 0:QC],
                                             start=(gi == 0), stop=(gi == DT - 1))
                            nc.tensor.matmul(po2[:], fT[:, gi, :], wc_sb[:, gi, QC:D],
                                             start=(gi == 0), stop=(gi == DT - 1))
                        o_sb = pf.tile([P, D], f32, tag="o_sb", bufs=3)
                        nc.vector.tensor_add(o_sb[:, 0:QC], po[:], bcb[:, 0:QC])
                        nc.vector.tensor_add(o_sb[:, QC:D], po2[:], bcb[:, QC:D])
                        nc.sync.dma_start(out[t * P:(t + 1) * P, :], o_sb[:])

    return nc


def _get_nc():
    global _nc
    if _nc is None:
        _nc = _build_program()
        _nc.finalize()
    return _nc


def _prep_in_maps(x, Wq, bq, Wk, bk, Wv, bv, Wc, bc):
    x = np.asarray(x, dtype=np.float32)
    wqT = np.ascontiguousarray(np.asarray(Wq, np.float32).T).astype(BF16)
    wkT = np.ascontiguousarray(np.asarray(Wk, np.float32).T).astype(BF16)
    wvT = np.ascontiguousarray(np.asarray(Wv, np.float32).T).astype(BF16)
    wcT = np.ascontiguousarray(np.asarray(Wc, np.float32).T).astype(BF16)
    bqc = np.asarray(bq, np.float32).reshape(D, 1).copy()
    bkc = np.asarray(bk, np.float32).reshape(D, 1).copy()
    bvc = np.asarray(bv, np.float32).reshape(D, 1).copy()
    bcc = np.asarray(bc, np.float32).reshape(1, D).copy()
    in_maps = []
    for c in range(8):
        b, h = divmod(c, 2)
        xb = x[b].astype(BF16)
        in_maps.append({
            "xq": xb,
            "xkv": np.ascontiguousarray(xb[h * SK:(h + 1) * SK]),
            "wqT": wqT, "wkT": wkT, "wvT": wvT, "wcT": wcT,
            "bq": bqc, "bk": bkc, "bv": bvc, "bc": bcc,
        })
    return in_maps


def _assemble(results):
    out = np.empty((B, S, D), dtype=np.float32)
    for c in range(8):
        b, h = divmod(c, 2)
        out[b, h * SK:(h + 1) * SK, :] = results[c]["out"]
    return out


def run_on_hw(trace=False, **inputs):
    from concourse.bass_utils import run_bass_kernel_spmd
    nc = _get_nc()
    in_maps = _prep_in_maps(**inputs)
    res = run_bass_kernel_spmd(nc, in_maps, list(range(8)), trace=trace)
    return _assemble(res.results), res


def kernel(**inputs):
    out, _ = run_on_hw(trace=False, **inputs)
    return out


# revision 13
# speedup vs baseline: 1.2088x; 1.0988x over previous
"""Trainium2 Bass kernel for single-head full-dim attention (nn_CasualSelfAttention).

Reference math (B=4, S=4096, D=768, fp32):
    q = x @ Wq.T + bq ; k = x @ Wk.T + bk ; v = x @ Wv.T + bv
    att = softmax(q @ k.T * D**-0.5)        # no mask
    y = att @ v
    y = y.transpose(0,2,1).reshape(B,S,D)   # element permutation
    out = y @ Wc.T + bc

Sharding (8 cores): core c = 2*b + h handles batch b with ALL 4096 queries but
only its half of the keys/values (rows h*2048:(h+1)*2048). Each core produces a
partial unnormalized yT [768, 4096] (features x queries) plus partial softmax
sums, with the value bias folded in linearly (bv x partial_sums). A pairwise
ReduceScatter(add) chunked by feature rows hands core h the fully-reduced
feature slice [384*h : 384*h+384] for all queries — exactly the rows of y.T
that the permutation maps to output rows [2048*h : 2048*h+2048]. After
normalizing by the (also-reduced) sums, the flat buffer IS y_perm row-major,
and the final projection runs locally. The RS is split into query-column
blocks (the last ones small) so communication overlaps attention compute and
the serial tail is short. Host pre-transposes/casts weights and activations to
bf16; matmuls are bf16 with fp32 accumulation, softmax in fp32 (logits are
bounded ~|1.8| so no max subtraction is needed).
"""

import numpy as np
import ml_dtypes

BF16 = ml_dtypes.bfloat16

B, S, D = 4, 4096, 768
SK = S // 2            # keys per core
P = 128
DT = D // P            # 6 feature tiles
KT = SK // P           # 16 key tiles
QC = 512               # query chunk width
NQC = S // QC          # 8 query chunks
BLOCKS = [(i, 1) for i in range(8)]   # RS blocks as (start_qc, n_qc)
FH = D // 2            # 384: feature rows per RS chunk
SCALE = float(D) ** -0.5
GROUPS = [[0, 1], [2, 3], [4, 5], [6, 7]]

_nc = None


def _build_program(phases="ABCDEF"):
    import concourse.bass as bass
    import concourse.mybir as mybir
    import concourse.tile as tile
    from concourse import bacc

    f32 = mybir.dt.float32
    bf16 = mybir.dt.bfloat16
    Exp = mybir.ActivationFunctionType.Exp
    mult = mybir.AluOpType.mult
    add = mybir.AluOpType.add

    qc2blk = {}
    for bi, (s0, n) in enumerate(BLOCKS):
        for j in range(n):
            qc2blk[s0 + j] = (bi, j)

    nc = bacc.Bacc(None, num_devices=8)

    xq = nc.declare_dram_parameter("xq", [S, D], bf16, isOutput=False)
    xkv = nc.declare_dram_parameter("xkv", [SK, D], bf16, isOutput=False)
    wqT = nc.declare_dram_parameter("wqT", [D, D], bf16, isOutput=False)
    wkT = nc.declare_dram_parameter("wkT", [D, D], bf16, isOutput=False)
    wvT = nc.declare_dram_parameter("wvT", [D, D], bf16, isOutput=False)
    wcT = nc.declare_dram_parameter("wcT", [D, D], bf16, isOutput=False)
    bq = nc.declare_dram_parameter("bq", [D, 1], f32, isOutput=False)
    bk = nc.declare_dram_parameter("bk", [D, 1], f32, isOutput=False)
    bv = nc.declare_dram_parameter("bv", [D, 1], f32, isOutput=False)
    bc = nc.declare_dram_parameter("bc", [1, D], f32, isOutput=False)
    out = nc.declare_dram_parameter("out", [SK, D], f32, isOutput=True)

    def wload(dst, src):
        # [768, 768] row-major -> [128, 6, 768] with logical row g*128+p
        nc.sync.dma_start(dst[:], src[:].rearrange("(g p) d -> p g d", p=P))

    with tile.TileContext(nc) as tc:
        with tc.tile_pool(name="persist", bufs=1) as pp, \
             tc.tile_pool(name="dram", bufs=1, space="DRAM") as dram:
            # Per column block: rows 0:384 = feats 0:384, row 384 = partial
            # sums, rows 385:769 = feats 384:768, row 769 = partial sums.
            yTaug = [dram.tile([2 * (FH + 1), n * QC], f32, name=f"yTaug{b}", tag=f"yTaug{b}")
                     for b, (_, n) in enumerate(BLOCKS)]
            rs_out = [dram.tile([FH + 1, n * QC], f32, name=f"rs_out{b}", tag=f"rs_out{b}")
                      for b, (_, n) in enumerate(BLOCKS)]
            f_dram = dram.tile([SK, D], bf16)

            # persistent SBUF: kT/qT/v activations + biases + output weights
            kT_sb = [pp.tile([P, SK], bf16, name=f"kT{g}", tag=f"kT{g}") for g in range(DT)]
            qT_sb = [pp.tile([P, S], bf16, name=f"qT{g}", tag=f"qT{g}") for g in range(DT)]
            v_sb = [pp.tile([P, D], bf16, name=f"v{t}", tag=f"v{t}") for t in range(KT)]
            bq_sb = [pp.tile([P, 1], f32, name=f"bq{g}", tag=f"bq{g}") for g in range(DT)]
            bk_sb = [pp.tile([P, 1], f32, name=f"bk{g}", tag=f"bk{g}") for g in range(DT)]
            bv_sb = [pp.tile([P, 1], f32, name=f"bv{g}", tag=f"bv{g}") for g in range(DT)]
            ones_sb = pp.tile([P, P], f32, name="ones", tag="ones")
            nc.vector.memset(ones_sb[:], 1.0)
            for g in range(DT):
                nc.sync.dma_start(bq_sb[g][:], bq[g * P:(g + 1) * P, :])
                nc.sync.dma_start(bk_sb[g][:], bk[g * P:(g + 1) * P, :])
                nc.sync.dma_start(bv_sb[g][:], bv[g * P:(g + 1) * P, :])
            wc_sb = pp.tile([P, DT, D], bf16, tag="wc_sb")
            wload(wc_sb, wcT)
            bc_sb = pp.tile([1, D], f32, tag="bc_sb")
            nc.sync.dma_start(bc_sb[:], bc[:])
            bcb = pp.tile([P, D], f32, tag="bcb")
            nc.gpsimd.partition_broadcast(bcb[:], bc_sb[:])

            # ---- Phase A: kT [768, 2048] and v [2048, 768] from xkv ----
            if "A" in phases:
                import contextlib
                _ab_stack = contextlib.ExitStack()
                pa = _ab_stack.enter_context(tc.tile_pool(name="pA", bufs=1))
                with tc.tile_pool(name="psA", bufs=2, space="PSUM") as psa:
                    wk_sb = pa.tile([P, DT, D], bf16, tag="wk_sb")
                    wload(wk_sb, wkT)
                    wv_sb = pa.tile([P, DT, D], bf16, tag="wv_sb")
                    wload(wv_sb, wvT)
                    for c in range(SK // QC):
                        xkvT = pa.tile([P, DT, QC], bf16, tag="xkvT", bufs=2, name="xkvT")
                        nc.sync.dma_start_transpose(xkvT[:], xkv[c * QC:(c + 1) * QC, :])
                        for go in range(DT):
                            ps = psa.tile([P, QC], f32, tag="pk")
                            for gi in range(DT):
                                nc.tensor.matmul(
                                    ps[:], wk_sb[:, gi, go * P:(go + 1) * P],
                                    xkvT[:, gi, :],
                                    start=(gi == 0), stop=(gi == DT - 1))
                            nc.vector.tensor_scalar_add(
                                kT_sb[go][:, c * QC:(c + 1) * QC], ps[:], bk_sb[go][:])
                        for tl in range(4):
                            t = c * 4 + tl
                            for half in range(2):
                                ps = psa.tile([P, FH], f32, tag="pv")
                                for gi in range(DT):
                                    nc.tensor.matmul(
                                        ps[:], xkvT[:, gi, tl * P:(tl + 1) * P],
                                        wv_sb[:, gi, half * FH:(half + 1) * FH],
                                        start=(gi == 0), stop=(gi == DT - 1))
                                nc.vector.tensor_copy(v_sb[t][:, half * FH:(half + 1) * FH], ps[:])

            # ---- Phase B: qT [768, 4096] from xq ----
            if "B" in phases:
                with tc.tile_pool(name="psB", bufs=2, space="PSUM") as psb:
                    pb = pa
                    wq_sb = pb.tile([P, DT, D], bf16, tag="wq_sb")
                    wload(wq_sb, wqT)
                    for c in range(NQC):
                        xqT = pb.tile([P, DT, QC], bf16, tag="xqT", bufs=3, name="xqT")
                        nc.sync.dma_start_transpose(xqT[:], xq[c * QC:(c + 1) * QC, :])
                        for go in range(DT):
                            ps = psb.tile([P, QC], f32, tag="pq")
                            for gi in range(DT):
                                nc.tensor.matmul(
                                    ps[:], wq_sb[:, gi, go * P:(go + 1) * P],
                                    xqT[:, gi, :],
                                    start=(gi == 0), stop=(gi == DT - 1))
                            nc.vector.tensor_scalar_add(
                                qT_sb[go][:, c * QC:(c + 1) * QC], ps[:], bq_sb[go][:])
                _ab_stack.close()

            # ---- Phase C: attention; write yTaug; chunked RS + normalize ----
            if "C" in phases:
                with tc.tile_pool(name="pC", bufs=2) as pc, \
                     tc.tile_pool(name="pE", bufs=2) as pe, \
                     tc.tile_pool(name="psC", bufs=1, space="PSUM") as psc:
                    f_view = f_dram[:].rearrange("a b -> (a b)").rearrange(
                        "(x c) -> x c", c=S)

                    MAXBW = max(n for _, n in BLOCKS) * QC

                    def emit_norm(b):
                        bw = BLOCKS[b][1] * QC
                        c0 = BLOCKS[b][0] * QC
                        s_row = pe.tile([1, MAXBW], f32, tag="s_row", name="s_row")
                        nc.sync.dma_start(s_row[:, :bw], rs_out[b][FH:FH + 1, :])
                        rec = pe.tile([1, MAXBW], f32, tag="rec", name="rec")
                        nc.vector.reciprocal(rec[:, :bw], s_row[:, :bw])
                        rbc = pe.tile([P, MAXBW], f32, tag="rbc", name="rbc")
                        nc.gpsimd.partition_broadcast(rbc[:, :bw], rec[:, :bw])
                        for r in range(FH // P):
                            fr = pe.tile([P, MAXBW], f32, tag="fr", bufs=2, name="fr")
                            nc.sync.dma_start(fr[:, :bw], rs_out[b][r * P:(r + 1) * P, :])
                            fn = pe.tile([P, MAXBW], bf16, tag="fn", bufs=2, name="fn")
                            nc.vector.tensor_mul(fn[:, :bw], fr[:, :bw], rbc[:, :bw])
                            nc.sync.dma_start(
                                f_view[r * P:(r + 1) * P, c0:c0 + bw], fn[:, :bw])

                    for qc in range(NQC):
                        blk, col = qc2blk[qc]
                        sums_acc = pc.tile([P, QC], f32, tag="sums_acc")
                        nc.vector.memset(sums_acc[:], 0.0)
                        ypsum = [psc.tile([P, QC], f32, name=f"y{e}", tag=f"y{e}", bufs=1)
                                 for e in range(DT)]
                        a_tiles = {}
                        for kt in range(KT):
                            aps = psc.tile([P, QC], f32, tag="att", bufs=2)
                            for gi in range(DT):
                                nc.tensor.matmul(
                                    aps[:], kT_sb[gi][:, kt * P:(kt + 1) * P],
                                    qT_sb[gi][:, qc * QC:(qc + 1) * QC],
                                    start=(gi == 0), stop=(gi == DT - 1))
                            # software pipeline: y-matmuls for kt-1 issue while
                            # the exp for kt is still on the scalar engine
                            if kt > 0:
                                for e in range(DT):
                                    nc.tensor.matmul(
                                        ypsum[e][:], v_sb[kt - 1][:, e * P:(e + 1) * P],
                                        a_tiles[kt - 1][:],
                                        start=(kt - 1 == 0), stop=False)
                            a_sb = pc.tile([P, QC], bf16, tag="a_sb", bufs=4)
                            a_tiles[kt] = a_sb
                            nc.scalar.activation(a_sb[:], aps[:], Exp, scale=SCALE)
                            nc.vector.tensor_add(sums_acc[:], sums_acc[:], a_sb[:])
                        for e in range(DT):
                            nc.tensor.matmul(
                                ypsum[e][:], v_sb[KT - 1][:, e * P:(e + 1) * P],
                                a_tiles[KT - 1][:],
                                start=False, stop=True)
                        # ones.T @ sums_acc both reduces across partitions and
                        # replicates the result onto all 128 partitions
                        sp = psc.tile([P, QC], f32, tag="att", bufs=2)
                        nc.tensor.matmul(sp[:], ones_sb[:], sums_acc[:], start=True, stop=True)
                        sbc = pc.tile([P, QC], f32, tag="sbc")
                        nc.vector.tensor_copy(sbc[:], sp[:])
                        yb = yTaug[blk]
                        nc.sync.dma_start(yb[FH:FH + 1, col * QC:(col + 1) * QC], sbc[0:1, :])
                        nc.sync.dma_start(yb[2 * FH + 1:2 * FH + 2, col * QC:(col + 1) * QC], sbc[0:1, :])
                        for e in range(DT):
                            yt_sb = pc.tile([P, QC], f32, tag="yt_sb", bufs=3)
                            # (sums_bcast * bv[e]) + ypsum  — folds the value bias
                            nc.vector.scalar_tensor_tensor(
                                yt_sb[:], sbc[:], bv_sb[e][:], ypsum[e][:], mult, add)
                            row = e * P if e < 3 else (FH + 1) + (e - 3) * P
                            nc.sync.dma_start(
                                yb[row:row + P, col * QC:(col + 1) * QC], yt_sb[:])

                        if col == BLOCKS[blk][1] - 1 and "D" in phases:
                            # block complete: reduce-scatter it. Normalization
                            # of block b-1 is emitted here (one block late) so
                            # nothing queues up behind an in-flight collective.
                            nc.gpsimd.collective_compute(
                                "ReduceScatter", mybir.AluOpType.add,
                                replica_groups=GROUPS,
                                ins=[yTaug[blk].opt()], outs=[rs_out[blk].opt()])
                            if "E" in phases and blk > 0:
                                emit_norm(blk - 1)

                    if "D" in phases and "E" in phases:
                        emit_norm(len(BLOCKS) - 1)

            # ---- Phase F: out = y_perm @ Wc.T + bc ----
            if "F" in phases:
                with tc.tile_pool(name="pF", bufs=1) as pf, \
                     tc.tile_pool(name="psF", bufs=2, space="PSUM") as psf:
                    fTs = []
                    for t in range(SK // P):
                        fT = pf.tile([P, DT, P], bf16, name=f"fT{t}", tag=f"fT{t}")
                        nc.sync.dma_start_transpose(fT[:], f_dram[t * P:(t + 1) * P, :])
                        fTs.append(fT)
                    for t in range(SK // P):
                        fT = fTs[t]
                        po = psf.tile([P, QC], f32, tag="po")
                        po2 = psf.tile([P, D - QC], f32, tag="po2")
                        for gi in range(DT):
                            nc.tensor.matmul(po[:], fT[:, gi, :], wc_sb[:, gi, 0:QC],
                                             start=(gi == 0), stop=(gi == DT - 1))
                            nc.tensor.matmul(po2[:], fT[:, gi, :], wc_sb[:, gi, QC:D],
                                             start=(gi == 0), stop=(gi == DT - 1))
                        o_sb = pf.tile([P, D], f32, tag="o_sb", bufs=3)
                        nc.vector.tensor_add(o_sb[:, 0:QC], po[:], bcb[:, 0:QC])
                        nc.vector.tensor_add(o_sb[:, QC:D], po2[:], bcb[:, QC:D])
                        nc.sync.dma_start(out[t * P:(t + 1) * P, :], o_sb[:])

    return nc


def _get_nc():
    global _nc
    if _nc is None:
        _nc = _build_program()
        _nc.finalize()
    return _nc


def _prep_in_maps(x, Wq, bq, Wk, bk, Wv, bv, Wc, bc):
    x = np.asarray(x, dtype=np.float32)
    wqT = np.ascontiguousarray(np.asarray(Wq, np.float32).T).astype(BF16)
    wkT = np.ascontiguousarray(np.asarray(Wk, np.float32).T).astype(BF16)
    wvT = np.ascontiguousarray(np.asarray(Wv, np.float32).T).astype(BF16)
    wcT = np.ascontiguousarray(np.asarray(Wc, np.float32).T).astype(BF16)
    bqc = np.asarray(bq, np.float32).reshape(D, 1).copy()
    bkc = np.asarray(bk, np.float32).reshape(D, 1).copy()
    bvc = np.asarray(bv, np.float32).reshape(D, 1).copy()
    bcc = np.asarray(bc, np.float32).reshape(1, D).copy()
    in_maps = []
    for c in range(8):
        b, h = divmod(c, 2)
        xb = x[b].astype(BF16)
        in_maps.append({
            "xq": xb,
            "xkv": np.ascontiguousarray(xb[h * SK:(h + 1) * SK]),
            "wqT": wqT, "wkT": wkT, "wvT": wvT, "wcT": wcT,
            "bq": bqc, "bk": bkc, "bv": bvc, "bc": bcc,
        })
    return in_maps


def _assemble(results):
    out = np.empty((B, S, D), dtype=np.float32)
    for c in range(8):
        b, h = divmod(c, 2)
        out[b, h * SK:(h + 1) * SK, :] = results[c]["out"]
    return out


def run_on_hw(trace=False, **inputs):
    from concourse.bass_utils import run_bass_kernel_spmd
    nc = _get_nc()
    in_maps = _prep_in_maps(**inputs)
    res = run_bass_kernel_spmd(nc, in_maps, list(range(8)), trace=trace)
    return _assemble(res.results), res


def kernel(**inputs):
    out, _ = run_on_hw(trace=False, **inputs)
    return out
